# revision 1
# baseline (speedup 1.0000x reference)
"""Causal attention (B=4, S=4096, D=64, fp32) on 8 Trainium2 NeuronCores. v3.

Sharding: core = (batch b in 0..3) x (key-parity role r in 0..1).
BOTH cores of a batch cover all 4096 queries as 16 slots; slot j is the
query-block pair (2j, 2j+1) = query columns [256j, 256j+256). Slot j needs
key chunks kc = 0..2j+1 (chunks of 128 keys); role r handles the chunks
with kc % 2 == r, i.e. position i in 0..j maps to kc = 2i + r. Each core
therefore runs an IDENTICAL instruction stream of sum(j+1) = 136 score
chunks (vs 144 for the v1/v2 parity-of-query-blocks sharding), with the
role differences entirely in the data:
  - kt/va hold only the core's key parity, packed: SBUF chunk i = keys
    (2i+r)*128 .. +128.  (Half the K/V input per core.)
  - the causal mask hits exactly position i == j of every slot, with ONE
    constant pattern per core: role 0: [tri | ones], role 1: [zeros | tri].
    (One DVE multiply per slot instead of four.)
  - each core outputs 16 PARTIAL (numerator^T, denominator) accumulators
    [65, 256]; the host adds the two roles' partials, then normalizes.

Device kernel as in v2: transposed scores via lhsT=KTaug [65,128]
(row 64 = padding bias), rhs=QTaug [65,256] (row 64 = ones, Q pre-scaled
by 1/8), exp on ACT in big flat groups (PSUM pools of 4 and 3 banks used
alternately), PV accumulates O^T in a single shared PSUM bank ([65,512],
slots alternate halves), bf16 matmuls, fp32 PSUM.
"""

import sys

if "/opt/trn_rl_repo" not in sys.path:
    sys.path.insert(0, "/opt/trn_rl_repo")

import os
import numpy as np

import concourse.bass as bass
import concourse.mybir as mybir
import concourse.tile as tile
from concourse.bass_utils import run_bass_kernel_spmd

B, S, D = 4, 4096, 64
NCORES = 8
NSLOT = 16                 # query-block pairs per batch (256 queries each)
MM_DT = os.environ.get("ATT_MM_DTYPE", "bf16")
# The PE HAM clock gate needs ~3.4us of dense activity to reach 2.4 GHz,
# and half-rate "real" matmuls never trip it: pre-warm long enough that the
# gate flips BEFORE the first score matmul issues.
N_WARM = int(os.environ.get("ATT_WARM", "72"))
N_WARM_GROUPS = int(os.environ.get("ATT_WARM_GROUPS", "8"))
N_WARM_PER = int(os.environ.get("ATT_WARM_PER", "2"))
NEG = -1.0e10

# Group size schedule: alternating PSUM pools A (<=8 chunks) and B (<=6).
# Even index -> pool A, odd -> pool B. Sum must be 136. Head groups MUST be
# small: with full-size head groups the PE idles ~1.7us during the first
# big ACTIVATE (no PV work is pipelined yet), which drops the HAM clock
# gate back to half rate for the next ~20us (measured).
GROUP_SIZES = [4, 4, 6, 6] + [8, 6] * 7 + [8, 6, 3, 1]
assert sum(GROUP_SIZES) == 136
for _i, _g in enumerate(GROUP_SIZES):
    assert _g <= (8 if _i % 2 == 0 else 6)


def _split_drain_waits(nc, max_waits=1):
    """Walrus in this container rejects instructions carrying more than one
    sync wait; hoist extra waits onto preceding single-wait nops on the same
    engine (the engine blocks on each nop's wait in order, so semantics are
    preserved - ge-waits on monotonic semaphores commute)."""
    for f in nc.m.functions:
        for bb in f.blocks:
            new_list = []
            changed = False
            for inst in bb.instructions:
                si = inst.sync_info
                if (
                    type(inst).__name__ != "InstNoOp"
                    and si is not None
                    and si.on_wait
                    and len(si.on_wait) > max_waits
                ):
                    waits = list(si.on_wait)
                    for j, w in enumerate(waits[max_waits:]):
                        new_list.append(
                            mybir.InstNoOp(
                                name=f"{inst.name}-hw{j}",
                                sync_info=mybir.SyncInfo(on_wait=[w], on_update=[]),
                                bass_nofuse=True,
                                engine=inst.engine,
                            )
                        )
                    si.on_wait = waits[:max_waits]
                    changed = True
                new_list.append(inst)
            if changed:
                bb.instructions = new_list
    return nc


def build_nc():
    f32 = mybir.dt.float32
    mm_dt = {
        "bf16": mybir.dt.bfloat16,
        "f32r": mybir.dt.float32r,
        "f32": mybir.dt.float32,
    }[MM_DT]

    nc = bass.Bass()
    qt_d = nc.dram_tensor("qt", [65, 4096], mm_dt, kind="ExternalInput")
    kt_d = nc.dram_tensor("kt", [65, 2048], mm_dt, kind="ExternalInput")
    va_d = nc.dram_tensor("va", [128, 16, 65], mm_dt, kind="ExternalInput")
    cm_d = nc.dram_tensor("cm", [128, 256], mm_dt, kind="ExternalInput")
    # Paired outputs: one contiguous store per two slots.
    ot_d = nc.dram_tensor("ot", [NSLOT // 2, 65, 512], f32, kind="ExternalOutput")

    # Chunk boundaries aligned with the schedule's first-need times:
    # slot j first used at job j(j+1)/2; kt/va chunk i first used at job
    # i(i+3)/2. qt chunk 0 goes through the gpsimd queue so it transfers in
    # parallel with kt chunk 0 on the sync queue.
    KT_BOUNDS = [0, 512, 1024, 2048]              # packed key columns
    VA_BOUNDS = [0, 4, 8, 16]                     # packed key chunk index
    QT_BOUNDS = [0, 512, 1024, 2048, 3072, 4096]  # query columns

    with tile.TileContext(nc) as tc:
        with (
            tc.tile_pool(name="inputs", bufs=1) as inp,
            tc.tile_pool(name="pt", bufs=4) as ptp,
            tc.tile_pool(name="otsb", bufs=2) as otp,
            tc.tile_pool(name="warm", bufs=1) as wrm,
            tc.tile_pool(name="psA", bufs=1, space="PSUM") as pspA,
            tc.tile_pool(name="psB", bufs=1, space="PSUM") as pspB,
            tc.tile_pool(name="ops", bufs=1, space="PSUM") as opp,
        ):
            # Warm the ACT exp table while DMAs run; memset off-Scalar so the
            # table load issues immediately.
            w = wrm.tile([128, 1], f32)
            nc.gpsimd.memset(w[:], 0.0)
            nc.scalar.activation(w[:], w[:], mybir.ActivationFunctionType.Exp)

            dummy = wrm.tile([128, 256], mm_dt)
            nc.gpsimd.memset(dummy[:], 0.0)

            # Single shared out bank: slots alternate halves [0:65, (j%2)*256].
            ob = opp.tile([128, 512], f32, tag="ops")

            # Pre-warm matmuls write a scratch region in the pool-B slot;
            # group 1's start=True score matmuls later overwrite it.
            wsink = pspB.tile([128, 64], f32, tag="ps1", name="wsink")

            def emit_warms(n):
                for _ in range(n):
                    nc.tensor.matmul(
                        wsink[:, 0:64], lhsT=dummy[:, :128], rhs=dummy[:, :64],
                        start=True, stop=True,
                    )

            emit_warms(N_WARM)

            qtt = [
                inp.tile([65, hi - lo], mm_dt, tag=f"qt{i}", name=f"qt{i}")
                for i, (lo, hi) in enumerate(zip(QT_BOUNDS, QT_BOUNDS[1:]))
            ]
            cm = inp.tile([128, 256], mm_dt, tag="cm")
            ktt = [
                inp.tile([65, hi - lo], mm_dt, tag=f"kt{i}", name=f"kt{i}")
                for i, (lo, hi) in enumerate(zip(KT_BOUNDS, KT_BOUNDS[1:]))
            ]
            vat = [
                inp.tile([128, hi - lo, 65], mm_dt, tag=f"va{i}", name=f"va{i}")
                for i, (lo, hi) in enumerate(zip(VA_BOUNDS, VA_BOUNDS[1:]))
            ]

            def load_kt(c, eng=None):
                lo, hi = KT_BOUNDS[c], KT_BOUNDS[c + 1]
                (eng or nc.sync).dma_start(ktt[c][:], kt_d[:, lo:hi])

            def load_va(c, eng=None):
                lo, hi = VA_BOUNDS[c], VA_BOUNDS[c + 1]
                (eng or nc.gpsimd).dma_start(vat[c][:], va_d[:, lo:hi, :])

            def load_qt(c, eng=None):
                lo, hi = QT_BOUNDS[c], QT_BOUNDS[c + 1]
                (eng or nc.sync).dma_start(qtt[c][:], qt_d[:, lo:hi])

            # Two DMA queues in parallel: sync (kt + qt1), gpsimd (qt0, cm,
            # va interleaved with the later qt chunks). Issue order follows
            # each chunk's first-need time.
            load_kt(0)
            load_qt(0, nc.gpsimd)
            load_qt(1)
            nc.gpsimd.dma_start(cm[:], cm_d[:])
            load_qt(2)
            load_va(0)
            load_kt(1)
            load_va(1)
            load_kt(2)
            load_qt(3, nc.gpsimd)
            load_va(2)
            load_qt(4, nc.gpsimd)

            def kt_ap(i):
                lo = i * 128
                for c in range(len(KT_BOUNDS) - 1):
                    if KT_BOUNDS[c] <= lo < KT_BOUNDS[c + 1]:
                        o = lo - KT_BOUNDS[c]
                        return ktt[c][:, o : o + 128]

            def va_ap(i):
                for c in range(len(VA_BOUNDS) - 1):
                    if VA_BOUNDS[c] <= i < VA_BOUNDS[c + 1]:
                        return vat[c][:, i - VA_BOUNDS[c], :]

            def qs_ap(j):
                lo = j * 256
                for c in range(len(QT_BOUNDS) - 1):
                    if QT_BOUNDS[c] <= lo < QT_BOUNDS[c + 1]:
                        o = lo - QT_BOUNDS[c]
                        return qtt[c][:, o : o + 256]

            # jobs: slot-major, position i = packed key chunk index.
            jobs = [(j, i) for j in range(NSLOT) for i in range(j + 1)]
            groups = []
            pos = 0
            for gsz in GROUP_SIZES:
                groups.append(jobs[pos : pos + gsz])
                pos += gsz
            assert pos == len(jobs)

            def ob_ap(j):
                half = (j % 2) * 256
                return ob[0:65, half : half + 256]

            # PV emission lags TWO groups behind scores: after ACT(g-2)
            # frees a score pool slot, the in-order PE queue reaches
            # scores(g) immediately instead of first draining PV(g-2)
            # (which itself only becomes runnable when ACT(g-2) ends).
            pendings = []   # [(group, pt), ...] depth <= 2
            stage = {}      # paired output staging tiles

            def emit_pv(group, pt):
                for idx, (j, i) in enumerate(group):
                    nc.tensor.matmul(
                        ob_ap(j),
                        lhsT=va_ap(i),
                        rhs=pt[:, idx * 256 : (idx + 1) * 256],
                        start=(i == 0),
                        stop=(i == j),
                    )
                    if i == j:
                        pair = j // 2
                        if j % 2 == 0:
                            st = otp.tile([65, 512], f32, tag="ot", name=f"ot{pair}")
                            stage[pair] = st
                            nc.vector.tensor_copy(st[:, 0:256], ob_ap(j))
                            if pair == NSLOT // 2 - 1:
                                # store the first half of the final pair
                                # immediately to shorten the drain chain
                                nc.sync.dma_start(
                                    ot_d[pair][:, 0:256], st[:, 0:256]
                                )
                        else:
                            st = stage.pop(pair)
                            nc.vector.tensor_copy(st[:, 256:512], ob_ap(j))
                            if pair == NSLOT // 2 - 1:
                                nc.sync.dma_start(
                                    ot_d[pair][:, 256:512], st[:, 256:512]
                                )
                            else:
                                nc.sync.dma_start(ot_d[pair], st[:])

            for gidx, group in enumerate(groups):
                m = len(group)
                pool = pspA if gidx % 2 == 0 else pspB
                ps = pool.tile(
                    [128, m * 256], f32, tag=f"ps{gidx % 2}", name=f"ps{gidx}"
                )
                # Keep the PE HAM clock gate fed through the stall-prone
                # early groups: dummy matmuls into this group's first score
                # chunk, overwritten right after by the real start=True
                # matmul (safe on the in-order PE queue).
                if gidx < N_WARM_GROUPS:
                    for _ in range(N_WARM_PER):
                        nc.tensor.matmul(
                            ps[:, 0:64], lhsT=dummy[:, :128], rhs=dummy[:, :64],
                            start=True, stop=True,
                        )
                for idx, (j, i) in enumerate(group):
                    nc.tensor.matmul(
                        ps[:, idx * 256 : (idx + 1) * 256],
                        lhsT=kt_ap(i),
                        rhs=qs_ap(j),
                        start=True,
                        stop=True,
                    )
                pt = ptp.tile([128, m * 256], mm_dt, tag="pt", name=f"pt{gidx}")
                nc.scalar.activation(
                    pt[:], ps[:], mybir.ActivationFunctionType.Exp
                )
                for idx, (j, i) in enumerate(group):
                    if i == j:
                        nc.vector.tensor_tensor(
                            pt[:, idx * 256 : (idx + 1) * 256],
                            pt[:, idx * 256 : (idx + 1) * 256],
                            cm[:],
                            mybir.AluOpType.mult,
                        )
                if len(pendings) == 2:
                    emit_pv(*pendings.pop(0))
                pendings.append((group, pt))
            for p_ in pendings:
                emit_pv(*p_)

    if os.environ.get("ATT_NO_SPLIT") != "1":
        _split_drain_waits(nc)
    return nc


_NC_CACHE = {}


def _get_nc():
    key = (MM_DT, N_WARM)
    if key not in _NC_CACHE:
        _NC_CACHE[key] = build_nc()
    return _NC_CACHE[key]


def _host_inputs(query, key, value, mask):
    import ml_dtypes

    np_mm = ml_dtypes.bfloat16 if MM_DT == "bf16" else np.float32
    tri = np.where(
        np.arange(128)[:, None] <= np.arange(128)[None, :], 1.0, 0.0
    ).astype(np.float32)
    ones = np.ones((128, 128), dtype=np.float32)
    zeros = np.zeros((128, 128), dtype=np.float32)
    cms = [
        np.concatenate([tri, ones], axis=1),    # role 0
        np.concatenate([zeros, tri], axis=1),   # role 1
    ]
    in_maps = []
    for b in range(B):
        qtb = np.concatenate(
            [(0.125 * query[b]).T, np.zeros((1, S), dtype=np.float32)], axis=0
        ).astype(np.float32)
        ktb_full = np.concatenate(
            [key[b].T, np.zeros((1, S), dtype=np.float32)], axis=0
        )
        vab = (
            np.concatenate([value[b], np.ones((S, 1), dtype=np.float32)], axis=1)
            * mask[b][:, None]
        ).astype(np.float32)
        va3 = vab.reshape(32, 128, 65)  # [kc, p, d]
        kt3 = ktb_full.reshape(65, 32, 128)  # [d, kc, col]
        for r in range(2):
            ktb = np.ascontiguousarray(
                kt3[:, r::2, :].reshape(65, 2048)
            )
            vap = np.ascontiguousarray(va3[r::2].transpose(1, 0, 2))  # [128,16,65]
            in_maps.append(
                {
                    "qt": np.ascontiguousarray(qtb.astype(np_mm)),
                    "kt": ktb.astype(np_mm),
                    "va": vap.astype(np_mm),
                    "cm": np.ascontiguousarray(cms[r].astype(np_mm)),
                }
            )
    return in_maps


def kernel(query, key, value, mask, _run_kwargs=None):
    query = np.asarray(query, dtype=np.float32)
    key = np.asarray(key, dtype=np.float32)
    value = np.asarray(value, dtype=np.float32)
    mask = np.asarray(mask, dtype=np.float32)

    nc = _get_nc()
    in_maps = _host_inputs(query, key, value, mask)
    kw = dict(_run_kwargs or {})
    try:
        res = run_bass_kernel_spmd(nc, in_maps, core_ids=list(range(NCORES)), **kw)
    except Exception:
        res = run_bass_kernel_spmd(nc, in_maps, core_ids=list(range(NCORES)), **kw)

    out = np.empty((B, S, D), dtype=np.float32)
    for b in range(B):
        # [8, 65, 512] partials per role -> [65, 4096]
        o0 = np.concatenate(list(res.results[2 * b]["ot"]), axis=1)
        o1 = np.concatenate(list(res.results[2 * b + 1]["ot"]), axis=1)
        ot = o0.astype(np.float64) + o1.astype(np.float64)
        out[b] = (ot[:64] / ot[64:65]).T.astype(np.float32)
    if _run_kwargs is not None:
        kernel.last_result = res
    return out


if __name__ == "__main__":
    rng = np.random.default_rng(0)
    q = rng.normal(size=(B, S, D)).astype(np.float32)
    k = rng.normal(size=(B, S, D)).astype(np.float32)
    v = rng.normal(size=(B, S, D)).astype(np.float32)
    m = np.ones((B, S), dtype=np.float32)
    o = kernel(q, k, v, m)
    # cpu check
    import math
    sc = (q[0] @ k[0].T) / 8.0
    sc = sc - np.triu(np.ones((S, S), dtype=np.float32), 1) * 1e10
    p = np.exp(sc - sc.max(axis=-1, keepdims=True))
    p /= p.sum(axis=-1, keepdims=True)
    ref = p @ v[0]
    err = np.abs(ref - o[0]).max()
    print("out", o.shape, o.dtype, "max|out|", float(np.abs(o).max()), "err b0:", err)

